# revision 10
# baseline (speedup 1.0000x reference)
"""Trainium2 Bass kernel for the CurrentLIF recurrent spiking network.

Strategy: column-shard the 4096x4096 recurrent weight matrix across 8
NeuronCores (512 postsynaptic neurons each); the (1-beta)-prescaled
weights stay SBUF-resident for all 500 steps in float32r, so the PE
streams them at 1 column/cycle (4x the plain-fp32 rate) with ~1.5e-5
relative error - far inside the spike-flip tolerance of this net.
Each step: 32 chained fp32r matmuls (binary spike vector stationary, W
moving) accumulate the per-synapse-type drive in PSUM; the drive is
transposed to neuron-major layout via the PE transpose path; the LIF
state update runs on DVE in [128, 64] tiles with the decay/refractory
ops scheduled into the AllGather window; the new spikes are exchanged
RAW (binary bf16, 16KB/rank) with an 8-core AllGather into Shared-DRAM
ring buffers, scattered per rank-half, and expanded on-device into the
sign/type-masked float32r stationary (so the matmul can start on ranks
0-3 while ranks 4-7 are still landing). The feed-forward drive is
precomputed on the host and streamed.
"""
import numpy as np

import concourse.bass as bass
import concourse.bacc as bacc
import concourse.tile as tile
import concourse.mybir as mybir
import concourse.bass_utils as bass_utils

F32 = mybir.dt.float32
F32R = mybir.dt.float32r
BF16 = mybir.dt.bfloat16
U8 = mybir.dt.uint8
AL = mybir.AluOpType

DT = 1.0
BETA = float(np.float32(np.exp(-DT / 20.0)))
ALPHA0 = float(np.float32(np.exp(-DT / 5.0)))
ALPHA1 = float(np.float32(np.exp(-DT / 10.0)))
B = 16
NIN = 1024
N = 4096
NSH = 512
KC = 32

_CACHE = {}


def _build(T):
    nc = bacc.Bacc("TRN2", target_bir_lowering=False, debug=False, num_devices=8)
    Wt_d = nc.dram_tensor("wt", [128, KC * NSH], F32R, kind="ExternalInput")
    FF_d = nc.dram_tensor("ff", [T, 128, 64], F32, kind="ExternalInput")
    MS_d = nc.dram_tensor("ms", [128, 1024], F32, kind="ExternalInput")
    AJ_d = nc.dram_tensor("aj", [128, 128], F32, kind="ExternalInput")
    GZ_d = nc.dram_tensor("gz", [128, KC * 16], F32R, kind="ExternalInput")
    ID_d = nc.dram_tensor("ident", [32, 32], F32, kind="ExternalInput")
    out_d = nc.dram_tensor("out", [T, 128, 64], F32, kind="ExternalOutput")

    with tile.TileContext(nc) as tc:
        with tc.tile_pool(name="big", bufs=1) as big, \
             tc.tile_pool(name="state", bufs=1) as state, \
             tc.tile_pool(name="work", bufs=3) as work, \
             tc.tile_pool(name="ffp", bufs=5) as ffp, \
             tc.tile_pool(name="psA", bufs=2, space="PSUM") as psA, \
             tc.tile_pool(name="psB", bufs=2, space="PSUM") as psB, \
             tc.tile_pool(name="dram", bufs=1, space="DRAM") as dram:

            Wt = big.tile([128, KC * NSH], F32R, name="Wt")
            # raw gathered spikes + mask-expanded stationary, split in
            # rank-halves so the matmul starts on ranks 0-3 while 4-7 land
            Graw = [[big.tile([128, 16 * 16], U8, name=f"Gr{i}{h}",
                              tag=f"Gr{i}{h}") for h in range(2)]
                    for i in range(2)]
            G2 = [[big.tile([128, 16 * 32], F32R, name=f"G2{i}{h}",
                            tag=f"G2{i}{h}") for h in range(2)]
                  for i in range(2)]
            nc.sync.dma_start(Wt[:], Wt_d[:])
            for gp in G2:
                for g_ in gp:
                    nc.sync.dma_start(g_[:], GZ_d[:])

            MS = state.tile([128, 1024], F32, name="MS")
            AJ = state.tile([128, 128], F32, name="AJ")
            ident = state.tile([32, 32], F32, name="ident")
            nc.sync.dma_start(MS[:], MS_d[:])
            nc.sync.dma_start(AJ[:], AJ_d[:])
            nc.sync.dma_start(ident[:], ID_d[:])
            MSr = MS[:].rearrange("p (s c b) -> p s c b", s=2, c=32)

            # J layout: [128, (q,s,b)] matching the transposed drive
            J = state.tile([128, 128], F32, name="J")
            v = state.tile([128, 64], F32, name="v")
            refr = state.tile([128, 64], F32, name="refr")
            am = state.tile([128, 64], F32, name="am")
            ns = state.tile([128, 64], F32, name="ns")
            for t_ in (J, v, refr):
                nc.gpsimd.memset(t_[:], 0.0)
            nc.gpsimd.memset(am[:], 1.0)
            nc.gpsimd.memset(ns[:], 1.0)

            Jr = J[:].rearrange("p (q s b) -> p q s b", q=4, s=2)

            ag_in = [dram.tile([128, 4, 16], U8, name=f"agin{i}", tag=f"agin{i}")
                     for i in range(2)]

            PF = 4
            ff_tiles = {}
            for tpre in range(min(PF, T)):
                ft = ffp.tile([128, 64], F32, name=f"ff{tpre}", tag="ff")
                nc.sync.dma_start(ft[:], FF_d[:][tpre])
                ff_tiles[tpre] = ft

            for t in range(T):
                par = t % 2
                ff = ff_tiles.pop(t)

                # ---- ops overlapping the previous step's AllGather ----
                nc.vector.tensor_tensor(J[:], J[:], AJ[:], AL.mult)
                nc.vector.tensor_tensor(Jr[:, :, 0, :], Jr[:, :, 0, :],
                                        ff[:].rearrange("p (q b) -> p q b", q=4),
                                        AL.add)
                # v <- beta * v * (1 - s_prev)   (ns precomputed from s_prev)
                nc.vector.tensor_tensor(v[:], v[:], ns[:], AL.mult)

                # ---- recurrent drive: 32 chained fp32r matmuls ----
                dr = psA.tile([32, NSH], F32, name=f"dr{par}", tag=f"dr{par}")
                for k in range(KC):
                    gh = G2[par][k // 16][:, 32 * (k % 16):32 * (k % 16) + 32]
                    nc.tensor.matmul(dr[:], gh,
                                     Wt[:, NSH * k:NSH * k + NSH],
                                     start=(k == 0), stop=(k == KC - 1))
                drc = work.tile([32, NSH], F32, name="drc", tag="drc")
                nc.scalar.copy(drc[:], dr[:])
                tp = psB.tile([128, 128], F32, name=f"tp{par}", tag=f"tp{par}")
                for q in range(4):
                    nc.tensor.transpose(tp[:, 32 * q:32 * q + 32],
                                        drc[:, 128 * q:128 * q + 128], ident[:])
                # J += transposed drive (layouts match)
                nc.vector.tensor_tensor(J[:], J[:], tp[:], AL.add)

                # ---- membrane + threshold ----
                vq = v[:].rearrange("p (q b) -> p q b", q=4)
                nc.vector.tensor_tensor(vq, vq, Jr[:, :, 0, :], AL.add)
                nc.vector.tensor_tensor(vq, vq, Jr[:, :, 1, :], AL.add)
                nc.vector.tensor_tensor(v[:], v[:], am[:], AL.mult)
                s = work.tile([128, 64], F32, name="s", tag="s")
                nc.vector.tensor_scalar(s[:], v[:], 1.0, None, AL.is_gt)

                # ---- stage raw spikes (bf16) and exchange ----
                if t < T - 1:
                    sb = work.tile([128, 64], U8, name="sb", tag="sb")
                    nc.vector.tensor_scalar(sb[:], v[:], 1.0, None, AL.is_gt)
                    nc.sync.dma_start(ag_in[par][:],
                                      sb[:].rearrange("p (q b) -> p q b", q=4))
                    ago = dram.tile([8, 128, 4, 16], U8, name=f"agout{t}",
                                    tag="agout", bufs=2, addr_space="Shared")
                    nc.gpsimd.collective_compute(
                        "AllGather", AL.bypass, replica_groups=[list(range(8))],
                        ins=[ag_in[par].opt()], outs=[ago.opt()])
                    for h in range(2):
                        gr = Graw[1 - par][h]
                        nc.gpsimd.dma_start(
                            gr[:].rearrange("p (r x) -> p r x", r=4),
                            ago[4 * h:4 * h + 4].rearrange("r p q b -> p r (q b)"))
                        # expand raw spikes to sign/type-masked stationary
                        g2r = G2[1 - par][h][:].rearrange(
                            "p (c s b) -> p c s b", c=16, s=2)
                        grr = gr[:].rearrange("p (c b) -> p c b", c=16)
                        nc.vector.tensor_tensor(
                            g2r[:, :, 0, :], grr,
                            MSr[:, 0, 16 * h:16 * h + 16, :], AL.mult)
                        nc.vector.tensor_tensor(
                            g2r[:, :, 1, :], grr,
                            MSr[:, 1, 16 * h:16 * h + 16, :], AL.mult)

                nc.scalar.dma_start(out_d[:][t], s[:])

                # ---- post-spike state (overlaps the AllGather) ----
                nc.vector.tensor_scalar(ns[:], s[:], -BETA, BETA, AL.mult, AL.add)
                nc.vector.tensor_scalar(refr[:], refr[:], -0.5, 0.0, AL.add, AL.max)
                nc.vector.tensor_tensor(refr[:], refr[:], s[:], AL.add)
                nc.vector.tensor_scalar(am[:], refr[:], 0.0, None, AL.is_le)

                if t + PF < T:
                    ft = ffp.tile([128, 64], F32, name=f"ff{t+PF}", tag="ff")
                    nc.sync.dma_start(ft[:], FF_d[:][t + PF])
                    ff_tiles[t + PF] = ft
    nc.compile()
    # Alias the per-step Shared AllGather outputs onto an 8-slot ring.
    # Unique tensors satisfy the scheduler's single-writer check; the strict
    # serial step chain (AG -> scatter -> matmul -> spikes -> next AG) keeps
    # reuses >= 7 steps apart, so aliasing is race-free and the Shared
    # scratchpad shrinks from T*128KB to 1MB.
    AGO_BYTES = 8 * 128 * 4 * 16
    for alloc in nc.m.functions[0].allocations:
        try:
            ml = alloc.memorylocations[0]
        except Exception:
            continue
        if ml.name.startswith("agout"):
            step = int(ml.name.split("_")[0][5:])
            ml.addr = (step % 8) * AGO_BYTES
    nc.shared_dram_base = 8 * AGO_BYTES
    return nc


def _prep_inputs(input_spikes, W, W_FF, cell_type_indices, T):
    beta = np.float32(BETA)
    Wp = ((np.float32(1.0) - beta) * W.astype(np.float32)).astype(np.float32)
    WFFp = ((np.float32(1.0) - beta) * W_FF.astype(np.float32)).astype(np.float32)
    B_, Tf, NIN_ = input_spikes.shape
    sp = input_spikes.astype(np.float32).transpose(1, 0, 2).reshape(Tf * B_, NIN_)[:T * B_]
    ff_all = (sp @ WFFp).reshape(T, B_, N)
    cti = np.asarray(cell_type_indices).astype(np.int32)
    ident = np.eye(32, dtype=np.float32)
    aj = np.empty((128, 128), np.float32)
    ajr = aj.reshape(128, 4, 2, 16)
    ajr[:, :, 0, :] = np.float32(ALPHA0)
    ajr[:, :, 1, :] = np.float32(ALPHA1)
    gz = np.zeros((128, KC * 16), np.float32)
    # expanded sign/type masks: ms[p, s, k, b] = sign_s * (cti[pre(k,p)] == s)
    # pre(k, p) = 512*(k//4) + 128*(k%4) + p
    pre = (512 * (np.arange(KC)[:, None] // 4) + 128 * (np.arange(KC)[:, None] % 4)
           + np.arange(128)[None, :])          # [k, p]
    t_of_pre = cti[pre]                        # [k, p]
    ms = np.zeros((128, 2, KC, 16), np.float32)
    ms[:, 0, :, :] = (t_of_pre.T == 0)[:, :, None].astype(np.float32)
    ms[:, 1, :, :] = -(t_of_pre.T == 1)[:, :, None].astype(np.float32)
    ms = ms.reshape(128, 1024).copy()
    in_maps = []
    for c in range(8):
        Wc = Wp[:, 512 * c:512 * (c + 1)]
        Wt = Wc.reshape(32, 128, 512).transpose(1, 0, 2).reshape(128, 32 * 512).copy()
        ffc = ff_all[:, :, 512 * c:512 * (c + 1)]
        FF = ffc.reshape(T, B_, 4, 128).transpose(0, 3, 2, 1).reshape(T, 128, 64).copy()
        in_maps.append({"wt": Wt, "ff": FF, "ms": ms, "aj": aj,
                        "gz": gz, "ident": ident})
    return in_maps


def _assemble(results, T):
    cols = []
    for c in range(8):
        arr = results[c]["out"].reshape(T, 128, 4, 16)
        cols.append(arr.transpose(3, 0, 2, 1).reshape(B, T, 512))
    return np.concatenate(cols, axis=2).astype(np.float32)


def kernel(input_spikes, W, W_FF, cell_type_indices):
    T = int(input_spikes.shape[1])
    if T not in _CACHE:
        _CACHE[T] = _build(T)
    nc = _CACHE[T]
    in_maps = _prep_inputs(np.asarray(input_spikes), np.asarray(W),
                           np.asarray(W_FF), np.asarray(cell_type_indices), T)
    res = bass_utils.run_bass_kernel_spmd(nc, in_maps, core_ids=list(range(8)))
    return _assemble(res.results, T)
